# revision 45
# baseline (speedup 1.0000x reference)
"""NeighborAttention (GNN message passing) Trainium2 Bass kernel.

Strategy: edges sorted by center node on host, sharded across 8 cores at
node boundaries (each node's edges live on exactly one core, so no
cross-core reduction is needed). Per core, edges are packed into fixed
2048-edge chunks covering <=127 nodes (slot 127 reserved for dummy
padding edges).

Device pipeline per chunk (engine-balanced):
  PE:     stage A (fused [bias|value] layer-0, bf16), stage B (block-diag
          mid layer), per-tile tail transpose producing values+logits with
          edge-major partitions, one-hot scatter with the one-hot as the
          stationary operand (68-row moving), finale transpose + W_O.
  ActE:   silu at 1024-column granularity only (exp lives on DVE: mixing Exp
          and Silu on ActE forces a 1.3us activation-table reload per switch).
  DVE:    exp via Horner polynomial (b2_b dropped: softmax is invariant to
          per-head constants), ex*v broadcast multiply, reciprocal, finale
          copies.
  The one-hot scatter matrix is precomputed on host and streamed from HBM
  (bf16) — no on-device build. wv2_b is folded into a host-side constant
  added to every real node's output row (attention weights sum to one per
  head).
"""

import numpy as np
import ml_dtypes

import concourse.bass as bass
import concourse.bacc as bacc
import concourse.mybir as mybir
import concourse.tile as tile
from concourse.bass_utils import run_bass_kernel_spmd

F32 = mybir.dt.float32
BF16 = mybir.dt.bfloat16
AF = mybir.ActivationFunctionType
ALU = mybir.AluOpType

NUM_HIDDEN = 64
NUM_IN = 128
N_HEADS = 4
HEAD_D = 16
SCALE = 1.0 / 4.0  # 1/sqrt(HEAD_D)

N_CORES = 8
CH_E = 2048          # edges per chunk
TILE_E = 128         # edges per tile
TPC = CH_E // TILE_E  # tiles per chunk (16)
BLK = 512            # matmul moving-dim block
HBLK = 1024          # silu / psum half-chunk granularity
NSLOT = 128          # node slots per chunk (127 real + 1 dummy)
DUMMY = NSLOT - 1
EXW = NUM_HIDDEN + N_HEADS  # 68: values + per-head ex columns

# exp(x) Horner coefficients (degree 5); logits are small (|x| <~ 1)
_EXP_COEF = [1.0 / 120, 1.0 / 24, 1.0 / 6, 0.5, 1.0]

# bf16 weight pack column offsets
WB_CAT0 = 0            # [128, 128] layer0 from h_E -> [bias64 | v64]
WB_B0V = 128           # [64, 128]  layer0 from h_V[center] (bias cols only)
WB_MID = 256           # [128, 128] block-diag mid layer
WB_TV = 384            # [128, 64]  wv2 (rows 64:128)
WB_TL = 448            # [128, 4]   b2*SCALE (rows 0:64)
WB_ID = 452            # [128, 128] identity (transpose helper)
WB_WO = 580            # [64, 64]   W_O
WB_COLS = 644
# f32 pack: col 0 = [b0_b;wv0_b], col 1 = [b1_b;wv1_b]
WF_COLS = 2


def build_program(n_chunks: int):
    """Build the per-core Bass program (identical across cores)."""
    nc = bacc.Bacc(trn_type="TRN2", target_bir_lowering=False, debug=False,
                   num_devices=N_CORES)
    cht = n_chunks * CH_E

    he_t = nc.dram_tensor("he_t", [NUM_IN, cht], BF16, kind="ExternalInput").ap()
    hv_t = nc.dram_tensor("hv_t", [NUM_HIDDEN, cht], BF16,
                          kind="ExternalInput").ap()
    oh_t = nc.dram_tensor("oh_t", [TILE_E, n_chunks * TPC * NSLOT], BF16,
                          kind="ExternalInput").ap()
    wf = nc.dram_tensor("wf", [128, WF_COLS], F32, kind="ExternalInput").ap()
    wb = nc.dram_tensor("wb", [128, WB_COLS], BF16, kind="ExternalInput").ap()
    out = nc.dram_tensor("out", [NUM_HIDDEN, n_chunks * NSLOT], BF16,
                         kind="ExternalOutput").ap()

    with tile.TileContext(nc) as tc:
        with (
            tc.tile_pool(name="const", bufs=1) as cpool,
            tc.tile_pool(name="he", bufs=4) as he_pool,
            tc.tile_pool(name="hv", bufs=4) as hv_pool,
            tc.tile_pool(name="oh", bufs=4) as oh_pool,
            tc.tile_pool(name="xa", bufs=2) as xa_pool,
            tc.tile_pool(name="xb", bufs=2) as xb_pool,
            tc.tile_pool(name="lb", bufs=2) as lb_pool,
            tc.tile_pool(name="exv", bufs=2) as exv_pool,
            tc.tile_pool(name="fin", bufs=2) as fin_pool,
            tc.tile_pool(name="ab", bufs=2, space="PSUM") as ab_pool,
            tc.tile_pool(name="pv", bufs=2, space="PSUM") as pv_pool,
            tc.tile_pool(name="pl", bufs=1, space="PSUM") as pl_pool,
            tc.tile_pool(name="ac", bufs=1, space="PSUM") as ac_pool,
        ):
            # ---- constants ----
            cw = cpool.tile([128, WF_COLS], F32, tag="wf")
            nc.sync.dma_start(out=cw[:], in_=wf[:])
            c_b01 = cw[:, 0:1]
            c_b12 = cw[:, 1:2]
            cb = cpool.tile([128, WB_COLS], BF16, tag="wb")
            nc.sync.dma_start(out=cb[:], in_=wb[:])
            c_cat0 = cb[:, WB_CAT0:WB_CAT0 + 128]
            # padded to K=128 (rows 64:128 zero): K=64 matmuls run slower
            c_b0v = cb[:, WB_B0V:WB_B0V + 128]
            c_mid = cb[:, WB_MID:WB_MID + 128]
            c_wtv = cb[:, WB_TV:WB_TV + NUM_HIDDEN]
            c_wtl = cb[:, WB_TL:WB_TL + N_HEADS]
            c_id = cb[:, WB_ID:WB_ID + 128]
            c_wo = cb[0:NUM_HIDDEN, WB_WO:WB_WO + NUM_HIDDEN]

            # state carried across loop iterations for the staggered finale
            prev = {}

            # 2-chunk-deep input prefetch so stage-A never waits on HBM
            loaded = {}

            def load_chunk(ci):
                he = he_pool.tile([NUM_IN, CH_E], BF16, tag="he")
                nc.sync.dma_start(out=he[:],
                                  in_=he_t[:, ci * CH_E:(ci + 1) * CH_E])
                hv = hv_pool.tile([128, CH_E], BF16, tag="hv")
                nc.sync.dma_start(out=hv[0:NUM_HIDDEN, :],
                                  in_=hv_t[:, ci * CH_E:(ci + 1) * CH_E])
                # zero the padding rows so 0-weights never meet NaN garbage
                nc.gpsimd.memset(hv[NUM_HIDDEN:128, :], 0.0)
                oh = oh_pool.tile([TILE_E, TPC * NSLOT], BF16, tag="oh")
                nc.sync.dma_start(
                    out=oh[:],
                    in_=oh_t[:, ci * TPC * NSLOT:(ci + 1) * TPC * NSLOT])
                loaded[ci] = (he, hv, oh)

            for ci in range(min(3, n_chunks)):
                load_chunk(ci)

            def emit_finale_div(acc):
                """Finale part 1 (DVE): attn = num/den for chunk c."""
                rec = fin_pool.tile([NSLOT, N_HEADS], F32, tag="rec")
                nc.vector.reciprocal_approx_fast(
                    out=rec[:], in_=acc[:, NUM_HIDDEN:EXW])
                at = fin_pool.tile([NSLOT, NUM_HIDDEN], BF16, tag="at")
                nc.vector.tensor_tensor(
                    at[:].rearrange("p (h d) -> p h d", h=N_HEADS),
                    acc[:, 0:NUM_HIDDEN].rearrange("p (h d) -> p h d",
                                                   h=N_HEADS),
                    rec[:].rearrange("p h -> p h ()").broadcast_to(
                        [NSLOT, N_HEADS, HEAD_D]),
                    op=ALU.mult)
                return at

            def emit_finale_out(c, at):
                """Finale part 2 (PE-heavy): transpose, W_O, store."""
                psT = pv_pool.tile([NUM_HIDDEN, NSLOT], BF16, tag="pv",
                                   name="psT")
                nc.tensor.transpose(psT[:], at[:], c_id)
                atT = fin_pool.tile([NUM_HIDDEN, NSLOT], BF16, tag="atT")
                # copies on ActE: Copy is in every act table set (no reload),
                # Scalar's idle window is right here, and DVE stays clear for
                # the scatter-critical poly/exv chain
                nc.scalar.copy(atT[:], psT[:])
                po = pv_pool.tile([NUM_HIDDEN, NSLOT], F32, tag="pv",
                                  name="po")
                nc.tensor.matmul(po[:], c_wo, atT[:], start=True, stop=True)
                so = fin_pool.tile([NUM_HIDDEN, NSLOT], BF16, tag="so")
                nc.scalar.copy(so[:], po[:])
                nc.sync.dma_start(out=out[:, c * NSLOT:(c + 1) * NSLOT],
                                  in_=so[:])

            def emit_ab_stage(ci):
                """Stages A+B for chunk ci: MLP layers 0 and 1 -> xb."""
                he, hv, _ = loaded[ci]
                xb = xb_pool.tile([128, CH_E], BF16, tag="xb")
                psA = []
                for h in range(CH_E // HBLK):
                    ps = ab_pool.tile([128, HBLK], F32, tag="ab",
                                      name=f"psA{h}")
                    psA.append(ps)
                    for s in range(HBLK // BLK):
                        lo = h * HBLK + s * BLK
                        nc.tensor.matmul(ps[:, s * BLK:(s + 1) * BLK],
                                         c_cat0, he[:, lo:lo + BLK],
                                         start=True, stop=False)
                        nc.tensor.matmul(ps[:, s * BLK:(s + 1) * BLK],
                                         c_b0v, hv[:, lo:lo + BLK],
                                         start=False, stop=True)
                xa = []
                for h in range(CH_E // HBLK):
                    x = xa_pool.tile([128, HBLK], BF16, tag="xa",
                                     name=f"xa{h}")
                    xa.append(x)
                    nc.scalar.activation(x[:], psA[h][:], AF.Silu, bias=c_b01)
                psB = []
                for h in range(CH_E // HBLK):
                    ps = ab_pool.tile([128, HBLK], F32, tag="ab",
                                      name=f"psB{h}")
                    psB.append(ps)
                    for s in range(HBLK // BLK):
                        nc.tensor.matmul(ps[:, s * BLK:(s + 1) * BLK],
                                         c_mid, xa[h][:, s * BLK:(s + 1) * BLK],
                                         start=True, stop=True)
                for h in range(CH_E // HBLK):
                    nc.scalar.activation(xb[:, h * HBLK:(h + 1) * HBLK],
                                         psB[h][:], AF.Silu, bias=c_b12)
                return xb

            for c in range(n_chunks):
                if c + 3 < n_chunks:
                    load_chunk(c + 3)
                xb = emit_ab_stage(c)
                _, _, oh = loaded.pop(c)

                # ---- tail: per-tile transpose into edge-major values+logits
                pl = pl_pool.tile([TILE_E, TPC * N_HEADS], F32, tag="pl")
                pvs = []
                for h in range(CH_E // HBLK):
                    pv = pv_pool.tile([TILE_E, (TPC // 2) * NUM_HIDDEN], F32,
                                      tag="pv", name=f"pv{h}")
                    pvs.append(pv)
                    for ti in range(TPC // 2):
                        t = h * (TPC // 2) + ti
                        xbt = xb[:, t * TILE_E:(t + 1) * TILE_E]
                        nc.tensor.matmul(
                            pv[:, ti * NUM_HIDDEN:(ti + 1) * NUM_HIDDEN],
                            xbt, c_wtv, start=True, stop=True)
                        nc.tensor.matmul(
                            pl[:, t * N_HEADS:(t + 1) * N_HEADS],
                            xbt, c_wtl, start=True, stop=True)

                # ---- ex = exp(logits) via ping-pong Horner on DVE:
                # z = x*c7; z = (z + c_k)*x for k=6..1; ex = z + 1
                exv = exv_pool.tile([TILE_E, TPC * EXW], BF16, tag="exv")
                exv4 = exv[:].rearrange("p (t f) -> p t f", t=TPC)
                acc_a = lb_pool.tile([TILE_E, TPC * N_HEADS], F32, tag="za")
                acc_b = lb_pool.tile([TILE_E, TPC * N_HEADS], F32, tag="zb")
                nc.vector.tensor_scalar_mul(acc_a[:], pl[:], _EXP_COEF[0])
                cur, nxt = acc_a, acc_b
                for k in range(1, len(_EXP_COEF)):
                    nc.vector.scalar_tensor_tensor(
                        nxt[:], cur[:], _EXP_COEF[k], pl[:],
                        op0=ALU.add, op1=ALU.mult)
                    cur, nxt = nxt, cur
                nc.vector.tensor_scalar_add(
                    exv4[:, :, NUM_HIDDEN:EXW], cur[:], 1.0)

                # ---- exv values: ex (broadcast over head dim) * v ----
                # (must stay on DVE: GPSIMD cannot access PSUM)
                for h in range(CH_E // HBLK):
                    hs = slice(h * (TPC // 2), (h + 1) * (TPC // 2))
                    nc.vector.tensor_tensor(
                        exv4[:, hs, 0:NUM_HIDDEN].rearrange(
                            "p t (x d) -> p t x d", x=N_HEADS),
                        exv4[:, hs, NUM_HIDDEN:EXW].rearrange(
                            "p t x -> p t x ()").broadcast_to(
                                [TILE_E, TPC // 2, N_HEADS, HEAD_D]),
                        pvs[h][:].rearrange("p (t x d) -> p t x d",
                                            t=TPC // 2, x=N_HEADS),
                        op=ALU.mult)

                # ---- staggered finale (prev chunk): DVE divide here, after
                # poly/exv so the scatter-critical DVE chain runs first ----
                if prev:
                    prev["at"] = emit_finale_div(prev["acc"])

                # ---- scatter: acc[slot, 0:64]=num, [64:68]=den ----
                acc = ac_pool.tile([NSLOT, EXW], F32, tag="ac", name="acc")
                for t in range(TPC):
                    nc.tensor.matmul(acc[:],
                                     oh[:, t * NSLOT:(t + 1) * NSLOT],
                                     exv4[:, t, :],
                                     start=(t == 0), stop=(t == TPC - 1))

                # ---- staggered finale (prev chunk): PE transpose + W_O ----
                if prev:
                    emit_finale_out(prev["c"], prev["at"])
                prev = {"c": c, "acc": acc}

            prev["at"] = emit_finale_div(prev["acc"])
            emit_finale_out(prev["c"], prev["at"])

    nc.compile()
    return nc


def pack_all(center, N, n_cores=N_CORES):
    """Sort edges by center node, split into cores and chunks.

    Returns: order (edge permutation), per-core dicts with edge index arrays
    (padded with -1), seg slots, chunk node lists.
    """
    center = np.asarray(center).astype(np.int64)
    E = center.shape[0]
    order = np.argsort(center, kind="stable")
    counts = np.bincount(center, minlength=N)
    csum = np.cumsum(counts)
    # core boundaries at node granularity
    bounds = [0]
    for k in range(1, n_cores):
        b = int(np.searchsorted(csum, k * E / n_cores))
        bounds.append(min(max(b, bounds[-1]), N))
    bounds.append(N)

    cores = []
    for k in range(n_cores):
        lo_n, hi_n = bounds[k], bounds[k + 1]
        chunks = []  # list of (node_list, degree_list)
        cur_nodes, cur_deg, cur_edges = [], [], 0
        for n in range(lo_n, hi_n):
            d = int(counts[n])
            if d == 0:
                continue
            assert d <= CH_E, f"node {n} degree {d} exceeds chunk size"
            if cur_edges + d > CH_E or len(cur_nodes) >= NSLOT - 1:
                chunks.append((cur_nodes, cur_deg))
                cur_nodes, cur_deg, cur_edges = [], [], 0
            cur_nodes.append(n)
            cur_deg.append(d)
            cur_edges += d
        if cur_nodes:
            chunks.append((cur_nodes, cur_deg))
        cores.append({"chunks": chunks, "lo_n": lo_n})
    n_chunks = max(len(c["chunks"]) for c in cores)

    node_start = np.concatenate([[0], csum[:-1]])  # first sorted-edge idx per node
    per_core = []
    for k in range(n_cores):
        chunks = cores[k]["chunks"]
        eidx = np.full(n_chunks * CH_E, -1, dtype=np.int64)
        seg = np.full(n_chunks * CH_E, DUMMY, dtype=np.int32)
        chunk_nodes = []
        for ci, (nodes, degs) in enumerate(chunks):
            pos = ci * CH_E
            for si, (n, d) in enumerate(zip(nodes, degs)):
                s = int(node_start[n])
                eidx[pos:pos + d] = order[s:s + d]
                seg[pos:pos + d] = si
                pos += d
            chunk_nodes.append(np.array(nodes, dtype=np.int64))
        for ci in range(len(chunks), n_chunks):
            chunk_nodes.append(np.array([], dtype=np.int64))
        per_core.append({"eidx": eidx, "seg": seg, "chunk_nodes": chunk_nodes})
    return n_chunks, per_core


def make_weights(inp):
    """Host-folded weight packs (f32 consts + bf16 matmul pack)."""
    f32 = np.float32
    b0_w = np.asarray(inp["b0_w"], f32)
    p = np.zeros((128, WF_COLS), f32)
    p[:, 0] = np.concatenate(
        [np.asarray(inp["b0_b"], f32), np.asarray(inp["wv0_b"], f32)])
    p[:, 1] = np.concatenate(
        [np.asarray(inp["b1_b"], f32), np.asarray(inp["wv1_b"], f32)])

    q = np.zeros((128, WB_COLS), f32)
    q[:, WB_CAT0:WB_CAT0 + 64] = b0_w[64:192, :]
    q[:, WB_CAT0 + 64:WB_CAT0 + 128] = np.asarray(inp["wv0_w"], f32)
    q[0:64, WB_B0V:WB_B0V + 64] = b0_w[0:64, :]
    q[0:64, WB_MID:WB_MID + 64] = np.asarray(inp["b1_w"], f32)
    q[64:128, WB_MID + 64:WB_MID + 128] = np.asarray(inp["wv1_w"], f32)
    q[64:128, WB_TV:WB_TV + 64] = np.asarray(inp["wv2_w"], f32)
    q[0:64, WB_TL:WB_TL + 4] = np.asarray(inp["b2_w"], f32) * SCALE
    q[:, WB_ID:WB_ID + 128] = np.eye(128, dtype=f32)
    q[0:64, WB_WO:WB_WO + 64] = np.asarray(inp["wo_w"], f32)
    return {"wf": p, "wb": q.astype(ml_dtypes.bfloat16)}


def prepare(inp):
    """Host-side prep: sort/shard/pack edges, build per-core input maps."""
    h_V = np.asarray(inp["h_V"], np.float32)
    h_E = np.asarray(inp["h_E"], np.float32)
    center = np.asarray(inp["center_id"])
    N = h_V.shape[0]

    n_chunks, per_core = pack_all(center, N)
    weights = make_weights(inp)

    in_maps = []
    for k in range(N_CORES):
        pc = per_core[k]
        eidx = pc["eidx"]
        valid = eidx >= 0
        he = np.zeros((eidx.shape[0], NUM_IN), np.float32)
        he[valid] = h_E[eidx[valid]]
        hv = np.zeros((eidx.shape[0], NUM_HIDDEN), np.float32)
        hv[valid] = h_V[center[eidx[valid]]]
        # one-hot scatter matrix: oh[e, (c, t, s)] = (seg[c, t, e] == s)
        seg_r = pc["seg"].reshape(n_chunks, TPC, TILE_E)
        ohm = np.zeros((TILE_E, n_chunks, TPC, NSLOT), ml_dtypes.bfloat16)
        c_i, t_i, e_i = np.indices((n_chunks, TPC, TILE_E), sparse=False)
        ohm[e_i, c_i, t_i, seg_r] = 1.0
        m = {
            "he_t": np.ascontiguousarray(he.T).astype(ml_dtypes.bfloat16),
            "hv_t": np.ascontiguousarray(hv.T).astype(ml_dtypes.bfloat16),
            "oh_t": np.ascontiguousarray(
                ohm.reshape(TILE_E, n_chunks * TPC * NSLOT)),
        }
        m.update(weights)
        in_maps.append(m)
    return n_chunks, per_core, in_maps, N


def assemble(results, per_core, n_chunks, N, const_out):
    """Scatter per-(core, chunk) node rows back to the full [N, 64] output."""
    out = np.zeros((N, NUM_HIDDEN), np.float32)
    for k in range(N_CORES):
        buf = np.asarray(results[k]["out"], np.float32).reshape(
            NUM_HIDDEN, n_chunks, NSLOT)
        for ci, nodes in enumerate(per_core[k]["chunk_nodes"]):
            if nodes.size:
                out[nodes] = buf[:, ci, :nodes.size].T + const_out
    return out


def kernel(h_V, h_E, center_id, wv0_w, wv0_b, wv1_w, wv1_b, wv2_w, wv2_b,
           b0_w, b0_b, b1_w, b1_b, b2_w, b2_b, wo_w, trace=False):
    inp = dict(h_V=h_V, h_E=h_E, center_id=center_id, wv0_w=wv0_w, wv0_b=wv0_b,
               wv1_w=wv1_w, wv1_b=wv1_b, wv2_w=wv2_w, wv2_b=wv2_b, b0_w=b0_w,
               b0_b=b0_b, b1_w=b1_w, b1_b=b1_b, b2_w=b2_w, b2_b=b2_b, wo_w=wo_w)
    n_chunks, per_core, in_maps, N = prepare(inp)
    nc = build_program(n_chunks)
    res = run_bass_kernel_spmd(nc, in_maps, list(range(N_CORES)), trace=trace)
    # attention weights sum to one per head, so wv2_b contributes a constant
    # row through W_O; added here instead of on-device
    const_out = np.asarray(wv2_b, np.float32) @ np.asarray(wo_w, np.float32)
    out = assemble(res.results, per_core, n_chunks, N, const_out)
    kernel.last_result = res
    return out


# revision 46
# speedup vs baseline: 1.1695x; 1.1695x over previous
"""NeighborAttention (GNN message passing) Trainium2 Bass kernel.

Strategy: edges sorted by center node on host, sharded across 8 cores at
node boundaries (each node's edges live on exactly one core, so no
cross-core reduction is needed). Per core, edges are packed into fixed
2048-edge chunks covering <=127 nodes (slot 127 reserved for dummy
padding edges).

Device pipeline per chunk (engine-balanced):
  PE:     stage A (fused [bias|value] layer-0, bf16), stage B (block-diag
          mid layer), per-tile tail transpose producing values+logits with
          edge-major partitions, one-hot scatter with the one-hot as the
          stationary operand (68-row moving), finale transpose + W_O.
  ActE:   silu at 1024-column granularity only (exp lives on DVE: mixing Exp
          and Silu on ActE forces a 1.3us activation-table reload per switch).
  DVE:    exp via Horner polynomial (b2_b dropped: softmax is invariant to
          per-head constants), ex*v broadcast multiply, reciprocal, finale
          copies.
  The one-hot scatter matrix is precomputed on host and streamed from HBM
  (bf16) — no on-device build. wv2_b is folded into a host-side constant
  added to every real node's output row (attention weights sum to one per
  head).
"""

import numpy as np
import ml_dtypes

import concourse.bass as bass
import concourse.bacc as bacc
import concourse.mybir as mybir
import concourse.tile as tile
from concourse.bass_utils import run_bass_kernel_spmd

F32 = mybir.dt.float32
BF16 = mybir.dt.bfloat16
AF = mybir.ActivationFunctionType
ALU = mybir.AluOpType

NUM_HIDDEN = 64
NUM_IN = 128
N_HEADS = 4
HEAD_D = 16
SCALE = 1.0 / 4.0  # 1/sqrt(HEAD_D)

N_CORES = 8
CH_E = 2048          # edges per chunk
TILE_E = 128         # edges per tile
TPC = CH_E // TILE_E  # tiles per chunk (16)
BLK = 512            # matmul moving-dim block
HBLK = 1024          # silu / psum half-chunk granularity
NSLOT = 128          # node slots per chunk (127 real + 1 dummy)
DUMMY = NSLOT - 1
EXW = NUM_HIDDEN + N_HEADS  # 68: values + per-head ex columns

# exp(x) Horner coefficients (degree 5); logits are small (|x| <~ 1)
_EXP_COEF = [1.0 / 120, 1.0 / 24, 1.0 / 6, 0.5, 1.0]

# bf16 weight pack column offsets
WB_CAT0 = 0            # [128, 128] layer0 from h_E -> [bias64 | v64]
WB_B0V = 128           # [64, 128]  layer0 from h_V[center] (bias cols only)
WB_MID = 256           # [128, 128] block-diag mid layer
WB_TV = 384            # [128, 64]  wv2 (rows 64:128)
WB_TL = 448            # [128, 4]   b2*SCALE (rows 0:64)
WB_ID = 452            # [128, 128] identity (transpose helper)
WB_WO = 580            # [64, 64]   W_O
WB_COLS = 644
# f32 pack: col 0 = [b0_b;wv0_b], col 1 = [b1_b;wv1_b]
WF_COLS = 2


def build_program(n_chunks: int):
    """Build the per-core Bass program (identical across cores)."""
    nc = bacc.Bacc(trn_type="TRN2", target_bir_lowering=False, debug=False,
                   num_devices=N_CORES)
    cht = n_chunks * CH_E

    he_t = nc.dram_tensor("he_t", [NUM_IN, cht], BF16, kind="ExternalInput").ap()
    hv_t = nc.dram_tensor("hv_t", [NUM_HIDDEN, cht], BF16,
                          kind="ExternalInput").ap()
    oh_t = nc.dram_tensor("oh_t", [TILE_E, n_chunks * TPC * NSLOT], BF16,
                          kind="ExternalInput").ap()
    wf = nc.dram_tensor("wf", [128, WF_COLS], F32, kind="ExternalInput").ap()
    wb = nc.dram_tensor("wb", [128, WB_COLS], BF16, kind="ExternalInput").ap()
    out = nc.dram_tensor("out", [NUM_HIDDEN, n_chunks * NSLOT], BF16,
                         kind="ExternalOutput").ap()

    with tile.TileContext(nc) as tc:
        with (
            tc.tile_pool(name="const", bufs=1) as cpool,
            tc.tile_pool(name="he", bufs=4) as he_pool,
            tc.tile_pool(name="hv", bufs=4) as hv_pool,
            tc.tile_pool(name="oh", bufs=4) as oh_pool,
            tc.tile_pool(name="xa", bufs=2) as xa_pool,
            tc.tile_pool(name="xb", bufs=2) as xb_pool,
            tc.tile_pool(name="lb", bufs=2) as lb_pool,
            tc.tile_pool(name="exv", bufs=2) as exv_pool,
            tc.tile_pool(name="fin", bufs=2) as fin_pool,
            tc.tile_pool(name="ab", bufs=2, space="PSUM") as ab_pool,
            tc.tile_pool(name="pv", bufs=2, space="PSUM") as pv_pool,
            tc.tile_pool(name="pl", bufs=1, space="PSUM") as pl_pool,
            tc.tile_pool(name="ac", bufs=1, space="PSUM") as ac_pool,
        ):
            # ---- constants ----
            cw = cpool.tile([128, WF_COLS], F32, tag="wf")
            nc.sync.dma_start(out=cw[:], in_=wf[:])
            c_b01 = cw[:, 0:1]
            c_b12 = cw[:, 1:2]
            cb = cpool.tile([128, WB_COLS], BF16, tag="wb")
            nc.sync.dma_start(out=cb[:], in_=wb[:])
            c_cat0 = cb[:, WB_CAT0:WB_CAT0 + 128]
            # padded to K=128 (rows 64:128 zero): K=64 matmuls run slower
            c_b0v = cb[:, WB_B0V:WB_B0V + 128]
            c_mid = cb[:, WB_MID:WB_MID + 128]
            c_wtv = cb[:, WB_TV:WB_TV + NUM_HIDDEN]
            c_wtl = cb[:, WB_TL:WB_TL + N_HEADS]
            c_id = cb[:, WB_ID:WB_ID + 128]
            c_wo = cb[0:NUM_HIDDEN, WB_WO:WB_WO + NUM_HIDDEN]

            # state carried across loop iterations for the staggered finale
            prev = {}

            # 2-chunk-deep input prefetch so stage-A never waits on HBM
            loaded = {}

            def load_chunk(ci):
                he = he_pool.tile([NUM_IN, CH_E], BF16, tag="he")
                nc.sync.dma_start(out=he[:],
                                  in_=he_t[:, ci * CH_E:(ci + 1) * CH_E])
                hv = hv_pool.tile([128, CH_E], BF16, tag="hv")
                nc.sync.dma_start(out=hv[0:NUM_HIDDEN, :],
                                  in_=hv_t[:, ci * CH_E:(ci + 1) * CH_E])
                # zero the padding rows so 0-weights never meet NaN garbage
                nc.gpsimd.memset(hv[NUM_HIDDEN:128, :], 0.0)
                oh = oh_pool.tile([TILE_E, TPC * NSLOT], BF16, tag="oh")
                nc.sync.dma_start(
                    out=oh[:],
                    in_=oh_t[:, ci * TPC * NSLOT:(ci + 1) * TPC * NSLOT])
                loaded[ci] = (he, hv, oh)

            for ci in range(min(3, n_chunks)):
                load_chunk(ci)

            def emit_finale_div(acc):
                """Finale part 1 (DVE): attn = num/den for chunk c."""
                rec = fin_pool.tile([NSLOT, N_HEADS], F32, tag="rec")
                nc.vector.reciprocal_approx_fast(
                    out=rec[:], in_=acc[:, NUM_HIDDEN:EXW])
                at = fin_pool.tile([NSLOT, NUM_HIDDEN], BF16, tag="at")
                nc.vector.tensor_tensor(
                    at[:].rearrange("p (h d) -> p h d", h=N_HEADS),
                    acc[:, 0:NUM_HIDDEN].rearrange("p (h d) -> p h d",
                                                   h=N_HEADS),
                    rec[:].rearrange("p h -> p h ()").broadcast_to(
                        [NSLOT, N_HEADS, HEAD_D]),
                    op=ALU.mult)
                return at

            def emit_finale_out(c, at):
                """Finale part 2 (PE-heavy): transpose, W_O, store."""
                psT = pv_pool.tile([NUM_HIDDEN, NSLOT], BF16, tag="pv",
                                   name="psT")
                nc.tensor.transpose(psT[:], at[:], c_id)
                atT = fin_pool.tile([NUM_HIDDEN, NSLOT], BF16, tag="atT")
                nc.vector.tensor_copy(atT[:], psT[:])
                po = pv_pool.tile([NUM_HIDDEN, NSLOT], F32, tag="pv",
                                  name="po")
                nc.tensor.matmul(po[:], c_wo, atT[:], start=True, stop=True)
                so = fin_pool.tile([NUM_HIDDEN, NSLOT], BF16, tag="so")
                nc.vector.tensor_copy(so[:], po[:])
                nc.sync.dma_start(out=out[:, c * NSLOT:(c + 1) * NSLOT],
                                  in_=so[:])

            def emit_ab_stage(ci):
                """Stages A+B for chunk ci: MLP layers 0 and 1 -> xb."""
                he, hv, _ = loaded[ci]
                xb = xb_pool.tile([128, CH_E], BF16, tag="xb")
                psA = []
                for h in range(CH_E // HBLK):
                    ps = ab_pool.tile([128, HBLK], F32, tag="ab",
                                      name=f"psA{h}")
                    psA.append(ps)
                    for s in range(HBLK // BLK):
                        lo = h * HBLK + s * BLK
                        nc.tensor.matmul(ps[:, s * BLK:(s + 1) * BLK],
                                         c_cat0, he[:, lo:lo + BLK],
                                         start=True, stop=False)
                        nc.tensor.matmul(ps[:, s * BLK:(s + 1) * BLK],
                                         c_b0v, hv[:, lo:lo + BLK],
                                         start=False, stop=True)
                xa = []
                for h in range(CH_E // HBLK):
                    x = xa_pool.tile([128, HBLK], BF16, tag="xa",
                                     name=f"xa{h}")
                    xa.append(x)
                    nc.scalar.activation(x[:], psA[h][:], AF.Silu, bias=c_b01)
                psB = []
                for h in range(CH_E // HBLK):
                    ps = ab_pool.tile([128, HBLK], F32, tag="ab",
                                      name=f"psB{h}")
                    psB.append(ps)
                    for s in range(HBLK // BLK):
                        nc.tensor.matmul(ps[:, s * BLK:(s + 1) * BLK],
                                         c_mid, xa[h][:, s * BLK:(s + 1) * BLK],
                                         start=True, stop=True)
                for h in range(CH_E // HBLK):
                    nc.scalar.activation(xb[:, h * HBLK:(h + 1) * HBLK],
                                         psB[h][:], AF.Silu, bias=c_b12)
                return xb

            for c in range(n_chunks):
                if c + 3 < n_chunks:
                    load_chunk(c + 3)
                xb = emit_ab_stage(c)
                _, _, oh = loaded.pop(c)

                # ---- tail: per-tile transpose into edge-major values+logits
                pl = pl_pool.tile([TILE_E, TPC * N_HEADS], F32, tag="pl")
                pvs = []
                for h in range(CH_E // HBLK):
                    pv = pv_pool.tile([TILE_E, (TPC // 2) * NUM_HIDDEN], F32,
                                      tag="pv", name=f"pv{h}")
                    pvs.append(pv)
                    for ti in range(TPC // 2):
                        t = h * (TPC // 2) + ti
                        xbt = xb[:, t * TILE_E:(t + 1) * TILE_E]
                        nc.tensor.matmul(
                            pv[:, ti * NUM_HIDDEN:(ti + 1) * NUM_HIDDEN],
                            xbt, c_wtv, start=True, stop=True)
                        nc.tensor.matmul(
                            pl[:, t * N_HEADS:(t + 1) * N_HEADS],
                            xbt, c_wtl, start=True, stop=True)

                # ---- ex = exp(logits) via ping-pong Horner on DVE:
                # z = x*c7; z = (z + c_k)*x for k=6..1; ex = z + 1
                exv = exv_pool.tile([TILE_E, TPC * EXW], BF16, tag="exv")
                exv4 = exv[:].rearrange("p (t f) -> p t f", t=TPC)
                acc_a = lb_pool.tile([TILE_E, TPC * N_HEADS], F32, tag="za")
                acc_b = lb_pool.tile([TILE_E, TPC * N_HEADS], F32, tag="zb")
                nc.vector.tensor_scalar_mul(acc_a[:], pl[:], _EXP_COEF[0])
                cur, nxt = acc_a, acc_b
                for k in range(1, len(_EXP_COEF)):
                    nc.vector.scalar_tensor_tensor(
                        nxt[:], cur[:], _EXP_COEF[k], pl[:],
                        op0=ALU.add, op1=ALU.mult)
                    cur, nxt = nxt, cur
                nc.vector.tensor_scalar_add(
                    exv4[:, :, NUM_HIDDEN:EXW], cur[:], 1.0)

                # ---- exv values: ex (broadcast over head dim) * v ----
                # (must stay on DVE: GPSIMD cannot access PSUM)
                for h in range(CH_E // HBLK):
                    hs = slice(h * (TPC // 2), (h + 1) * (TPC // 2))
                    nc.vector.tensor_tensor(
                        exv4[:, hs, 0:NUM_HIDDEN].rearrange(
                            "p t (x d) -> p t x d", x=N_HEADS),
                        exv4[:, hs, NUM_HIDDEN:EXW].rearrange(
                            "p t x -> p t x ()").broadcast_to(
                                [TILE_E, TPC // 2, N_HEADS, HEAD_D]),
                        pvs[h][:].rearrange("p (t x d) -> p t x d",
                                            t=TPC // 2, x=N_HEADS),
                        op=ALU.mult)

                # ---- staggered finale (prev chunk): DVE divide here, after
                # poly/exv so the scatter-critical DVE chain runs first ----
                if prev:
                    prev["at"] = emit_finale_div(prev["acc"])

                # ---- scatter: acc[slot, 0:64]=num, [64:68]=den ----
                acc = ac_pool.tile([NSLOT, EXW], F32, tag="ac", name="acc")
                for t in range(TPC):
                    nc.tensor.matmul(acc[:],
                                     oh[:, t * NSLOT:(t + 1) * NSLOT],
                                     exv4[:, t, :],
                                     start=(t == 0), stop=(t == TPC - 1))

                # ---- staggered finale (prev chunk): PE transpose + W_O ----
                if prev:
                    emit_finale_out(prev["c"], prev["at"])
                prev = {"c": c, "acc": acc}

            prev["at"] = emit_finale_div(prev["acc"])
            emit_finale_out(prev["c"], prev["at"])

    nc.compile()
    return nc


def pack_all(center, N, n_cores=N_CORES):
    """Sort edges by center node, split into cores and chunks.

    Returns: order (edge permutation), per-core dicts with edge index arrays
    (padded with -1), seg slots, chunk node lists.
    """
    center = np.asarray(center).astype(np.int64)
    E = center.shape[0]
    order = np.argsort(center, kind="stable")
    counts = np.bincount(center, minlength=N)
    csum = np.cumsum(counts)
    # core boundaries at node granularity
    bounds = [0]
    for k in range(1, n_cores):
        b = int(np.searchsorted(csum, k * E / n_cores))
        bounds.append(min(max(b, bounds[-1]), N))
    bounds.append(N)

    cores = []
    for k in range(n_cores):
        lo_n, hi_n = bounds[k], bounds[k + 1]
        chunks = []  # list of (node_list, degree_list)
        cur_nodes, cur_deg, cur_edges = [], [], 0
        for n in range(lo_n, hi_n):
            d = int(counts[n])
            if d == 0:
                continue
            assert d <= CH_E, f"node {n} degree {d} exceeds chunk size"
            if cur_edges + d > CH_E or len(cur_nodes) >= NSLOT - 1:
                chunks.append((cur_nodes, cur_deg))
                cur_nodes, cur_deg, cur_edges = [], [], 0
            cur_nodes.append(n)
            cur_deg.append(d)
            cur_edges += d
        if cur_nodes:
            chunks.append((cur_nodes, cur_deg))
        cores.append({"chunks": chunks, "lo_n": lo_n})
    n_chunks = max(len(c["chunks"]) for c in cores)

    node_start = np.concatenate([[0], csum[:-1]])  # first sorted-edge idx per node
    per_core = []
    for k in range(n_cores):
        chunks = cores[k]["chunks"]
        eidx = np.full(n_chunks * CH_E, -1, dtype=np.int64)
        seg = np.full(n_chunks * CH_E, DUMMY, dtype=np.int32)
        chunk_nodes = []
        for ci, (nodes, degs) in enumerate(chunks):
            pos = ci * CH_E
            for si, (n, d) in enumerate(zip(nodes, degs)):
                s = int(node_start[n])
                eidx[pos:pos + d] = order[s:s + d]
                seg[pos:pos + d] = si
                pos += d
            chunk_nodes.append(np.array(nodes, dtype=np.int64))
        for ci in range(len(chunks), n_chunks):
            chunk_nodes.append(np.array([], dtype=np.int64))
        per_core.append({"eidx": eidx, "seg": seg, "chunk_nodes": chunk_nodes})
    return n_chunks, per_core


def make_weights(inp):
    """Host-folded weight packs (f32 consts + bf16 matmul pack)."""
    f32 = np.float32
    b0_w = np.asarray(inp["b0_w"], f32)
    p = np.zeros((128, WF_COLS), f32)
    p[:, 0] = np.concatenate(
        [np.asarray(inp["b0_b"], f32), np.asarray(inp["wv0_b"], f32)])
    p[:, 1] = np.concatenate(
        [np.asarray(inp["b1_b"], f32), np.asarray(inp["wv1_b"], f32)])

    q = np.zeros((128, WB_COLS), f32)
    q[:, WB_CAT0:WB_CAT0 + 64] = b0_w[64:192, :]
    q[:, WB_CAT0 + 64:WB_CAT0 + 128] = np.asarray(inp["wv0_w"], f32)
    q[0:64, WB_B0V:WB_B0V + 64] = b0_w[0:64, :]
    q[0:64, WB_MID:WB_MID + 64] = np.asarray(inp["b1_w"], f32)
    q[64:128, WB_MID + 64:WB_MID + 128] = np.asarray(inp["wv1_w"], f32)
    q[64:128, WB_TV:WB_TV + 64] = np.asarray(inp["wv2_w"], f32)
    q[0:64, WB_TL:WB_TL + 4] = np.asarray(inp["b2_w"], f32) * SCALE
    q[:, WB_ID:WB_ID + 128] = np.eye(128, dtype=f32)
    q[0:64, WB_WO:WB_WO + 64] = np.asarray(inp["wo_w"], f32)
    return {"wf": p, "wb": q.astype(ml_dtypes.bfloat16)}


def prepare(inp):
    """Host-side prep: sort/shard/pack edges, build per-core input maps."""
    h_V = np.asarray(inp["h_V"], np.float32)
    h_E = np.asarray(inp["h_E"], np.float32)
    center = np.asarray(inp["center_id"])
    N = h_V.shape[0]

    n_chunks, per_core = pack_all(center, N)
    weights = make_weights(inp)

    in_maps = []
    for k in range(N_CORES):
        pc = per_core[k]
        eidx = pc["eidx"]
        valid = eidx >= 0
        he = np.zeros((eidx.shape[0], NUM_IN), np.float32)
        he[valid] = h_E[eidx[valid]]
        hv = np.zeros((eidx.shape[0], NUM_HIDDEN), np.float32)
        hv[valid] = h_V[center[eidx[valid]]]
        # one-hot scatter matrix: oh[e, (c, t, s)] = (seg[c, t, e] == s)
        seg_r = pc["seg"].reshape(n_chunks, TPC, TILE_E)
        ohm = np.zeros((TILE_E, n_chunks, TPC, NSLOT), ml_dtypes.bfloat16)
        c_i, t_i, e_i = np.indices((n_chunks, TPC, TILE_E), sparse=False)
        ohm[e_i, c_i, t_i, seg_r] = 1.0
        m = {
            "he_t": np.ascontiguousarray(he.T).astype(ml_dtypes.bfloat16),
            "hv_t": np.ascontiguousarray(hv.T).astype(ml_dtypes.bfloat16),
            "oh_t": np.ascontiguousarray(
                ohm.reshape(TILE_E, n_chunks * TPC * NSLOT)),
        }
        m.update(weights)
        in_maps.append(m)
    return n_chunks, per_core, in_maps, N


def assemble(results, per_core, n_chunks, N, const_out):
    """Scatter per-(core, chunk) node rows back to the full [N, 64] output."""
    out = np.zeros((N, NUM_HIDDEN), np.float32)
    for k in range(N_CORES):
        buf = np.asarray(results[k]["out"], np.float32).reshape(
            NUM_HIDDEN, n_chunks, NSLOT)
        for ci, nodes in enumerate(per_core[k]["chunk_nodes"]):
            if nodes.size:
                out[nodes] = buf[:, ci, :nodes.size].T + const_out
    return out


def kernel(h_V, h_E, center_id, wv0_w, wv0_b, wv1_w, wv1_b, wv2_w, wv2_b,
           b0_w, b0_b, b1_w, b1_b, b2_w, b2_b, wo_w, trace=False):
    inp = dict(h_V=h_V, h_E=h_E, center_id=center_id, wv0_w=wv0_w, wv0_b=wv0_b,
               wv1_w=wv1_w, wv1_b=wv1_b, wv2_w=wv2_w, wv2_b=wv2_b, b0_w=b0_w,
               b0_b=b0_b, b1_w=b1_w, b1_b=b1_b, b2_w=b2_w, b2_b=b2_b, wo_w=wo_w)
    n_chunks, per_core, in_maps, N = prepare(inp)
    nc = build_program(n_chunks)
    res = run_bass_kernel_spmd(nc, in_maps, list(range(N_CORES)), trace=trace)
    # attention weights sum to one per head, so wv2_b contributes a constant
    # row through W_O; added here instead of on-device
    const_out = np.asarray(wv2_b, np.float32) @ np.asarray(wo_w, np.float32)
    out = assemble(res.results, per_core, n_chunks, N, const_out)
    kernel.last_result = res
    return out
